# revision 16
# baseline (speedup 1.0000x reference)
"""AdaProp GNN message-passing kernel for 8 TRN2 NeuronCores.

Strategy: shard edges by destination-node range (6250 nodes per core) so the
segment-sum is fully local per core (no all-reduce). Precompute
  hG   = [hidden @ Ws | hidden @ Wh]        [N, 256]  (sharded build + AllGather)
  hrG  = [rela  @ Wr | rela  @ Wh]          [401, 256]
  hqr  = rela[q_rel] @ Wqr_w + Wqr_b        [64, 128]
Then per edge only three row gathers are needed (dma_gather, int16 indices —
hence the hG table is gathered as two <32768-row halves); the attention logit
is a fused relu-mul-accumulate on DVE; the segment sum is a one-hot
(alpha-scaled) matmul accumulating in PSUM; Wh is folded into the tables; the
final relu rides the DVE PSUM eviction.

Hardware constraint baked throughout: this walrus build allows at most ONE
semaphore wait per PE instruction, so every tile read by the TensorEngine is
last-written by the DVE and PSUM slots are recycled by DVE readers.
"""

import numpy as np

N, E, B, D = 50000, 500_000, 64, 128
NCORES = 8
NPC = 6250            # output nodes per core
WIN = 128             # nodes per PSUM window
NWIN = (NPC + WIN - 1) // WIN          # 49 windows per core
TBL_ROWS = NWIN * WIN                  # 6272 rows per hG slice
TBL_FULL = TBL_ROWS * NCORES           # 50176
HALF = TBL_FULL // 2                   # 25088 (< 32768 so int16 indices work)
G = 3                 # windows per gather group
P = 128


def _host_shard(edges):
    sub = np.asarray(edges[:, 4], dtype=np.int64)
    rel = np.asarray(edges[:, 2], dtype=np.int64)
    obj = np.asarray(edges[:, 5], dtype=np.int64)
    ridx = np.asarray(edges[:, 0], dtype=np.int64)

    core = obj // NPC
    loc = obj - core * NPC
    win = loc // WIN
    sel = loc - win * WIN
    half = (sub >= HALF).astype(np.int64)

    # per (core, window, half) edge index lists
    lists = [[[None, None] for _ in range(NWIN)] for _ in range(NCORES)]
    for k in range(NCORES):
        mk = np.nonzero(core == k)[0]
        key = win[mk] * 2 + half[mk]
        order = np.argsort(key, kind="stable")
        mk = mk[order]
        key = key[order]
        bounds = np.searchsorted(key, np.arange(2 * NWIN + 1))
        for w in range(NWIN):
            lists[k][w][0] = mk[bounds[2 * w]:bounds[2 * w + 1]]
            lists[k][w][1] = mk[bounds[2 * w + 1]:bounds[2 * w + 2]]

    # global per-(window,half) tile counts -> identical SPMD graph on all cores
    tcA = [max(len(lists[k][w][0]) for k in range(NCORES)) for w in range(NWIN)]
    tcB = [max(len(lists[k][w][1]) for k in range(NCORES)) for w in range(NWIN)]
    tcA = [(n + P - 1) // P for n in tcA]
    tcB = [(n + P - 1) // P for n in tcB]
    for w in range(NWIN):
        if tcA[w] + tcB[w] == 0:
            tcA[w] = 1

    # groups of G windows; tile stream per group: [A tiles][B tiles]
    groups = []          # (c_start, tilesA, tilesB, windowsA list, windowsB list)
    tile_window = []
    c = 0
    for g0 in range(0, NWIN, G):
        ws = list(range(g0, min(g0 + G, NWIN)))
        tA = sum(tcA[w] for w in ws)
        tB = sum(tcB[w] for w in ws)
        for w in ws:
            tile_window += [w] * tcA[w]
        for w in ws:
            tile_window += [w] * tcB[w]
        groups.append((c, tA, tB))
        c += tA + tB
    ctot = c
    S = ctot * P // 16   # idx array columns

    subs16 = np.zeros((NCORES, 16, S), dtype=np.int16)
    rels16 = np.zeros((NCORES, 16, S), dtype=np.int16)
    rids16 = np.zeros((NCORES, 16, S), dtype=np.int16)
    objs = np.full((NCORES, P, ctot), -1.0, dtype=np.float32)

    for k in range(NCORES):
        gi = 0
        for g0 in range(0, NWIN, G):
            ws = list(range(g0, min(g0 + G, NWIN)))
            c_start, tA, tB = groups[gi]
            gi += 1
            s0 = c_start * P // 16        # idx column base of this group
            n_all = (tA + tB) * P

            # build the group's slot-ordered edge list (A runs then B runs)
            slot_sub = np.zeros(n_all, dtype=np.int64)
            slot_rel = np.zeros(n_all, dtype=np.int64)
            slot_rid = np.zeros(n_all, dtype=np.int64)
            slot_obj = np.full(n_all, -1.0, dtype=np.float32)
            pos = 0
            for h, tc in ((0, tcA), (1, tcB)):
                for w in ws:
                    idx = lists[k][w][h]
                    n = len(idx)
                    nt = tc[w] * P
                    if n:
                        slot_sub[pos:pos + n] = sub[idx]
                        slot_rel[pos:pos + n] = rel[idx]
                        slot_rid[pos:pos + n] = ridx[idx]
                        slot_obj[pos:pos + n] = sel[idx]
                    # pad slots: harmless gather target in the right half
                    slot_sub[pos + n:pos + nt] = 0 if h == 0 else HALF
                    pos += nt

            # per-slot arrays in [p, c] layout (slot j -> p=j%128, c=j//128)
            j = np.arange(n_all)
            objs[k, j % P, c_start + j // P] = slot_obj
            # idx arrays in 16-partition wrap per gather run
            nA = tA * P
            jA = np.arange(nA)
            subs16[k, jA % 16, s0 + jA // 16] = slot_sub[:nA]
            jB = np.arange(n_all - nA)
            subs16[k, jB % 16, s0 + nA // 16 + jB // 16] = slot_sub[nA:] - HALF
            rels16[k, j % 16, s0 + j // 16] = slot_rel
            rids16[k, j % 16, s0 + j // 16] = slot_rid

    subs16 = np.tile(subs16, (1, 8, 1))   # replicate into all 128 partitions
    rels16 = np.tile(rels16, (1, 8, 1))
    rids16 = np.tile(rids16, (1, 8, 1))
    return subs16, rels16, rids16, objs, tile_window, groups, ctot


def _build_graph(ctot, tile_window, groups):
    import concourse.bass as bass
    import concourse.bacc as bacc
    import concourse.mybir as mybir
    from concourse.tile import TileContext
    from concourse.masks import make_identity

    f32 = mybir.dt.float32
    bf16 = mybir.dt.bfloat16
    i16 = mybir.dt.int16
    AF = mybir.ActivationFunctionType
    Alu = mybir.AluOpType

    S = ctot * P // 16

    nc = bacc.Bacc()
    hid_s = nc.declare_dram_parameter("hid_s", [TBL_ROWS, D], f32, isOutput=False)
    rela = nc.declare_dram_parameter("rela", [401, D], f32, isOutput=False)
    qrel = nc.declare_dram_parameter("qrel", [64, D], f32, isOutput=False)
    ws = nc.declare_dram_parameter("ws", [D, D], f32, isOutput=False)
    wr = nc.declare_dram_parameter("wr", [D, D], f32, isOutput=False)
    wh = nc.declare_dram_parameter("wh", [D, D], f32, isOutput=False)
    wqr = nc.declare_dram_parameter("wqr", [D, D], f32, isOutput=False)
    wqrb = nc.declare_dram_parameter("wqrb", [1, D], f32, isOutput=False)
    wa = nc.declare_dram_parameter("wa", [1, D], f32, isOutput=False)
    sub_i = nc.declare_dram_parameter("sub_i", [P, S], i16, isOutput=False)
    rel_i = nc.declare_dram_parameter("rel_i", [P, S], i16, isOutput=False)
    rid_i = nc.declare_dram_parameter("rid_i", [P, S], i16, isOutput=False)
    obj_f = nc.declare_dram_parameter("obj_f", [P, ctot], f32, isOutput=False)
    out_ext = nc.declare_dram_parameter("out", [TBL_ROWS, D], f32, isOutput=True)

    first_tile = {}
    last_tile = {}
    for c, w in enumerate(tile_window):
        if w not in first_tile:
            first_tile[w] = c
        last_tile[w] = c

    with TileContext(nc) as tc:
        with (
            tc.tile_pool(name="const", bufs=1) as cpool,
            tc.tile_pool(name="dram", bufs=1, space="DRAM") as dpool,
            tc.tile_pool(name="work", bufs=2) as wpool,
            tc.tile_pool(name="psum", bufs=2, space="PSUM") as ppool,
            tc.tile_pool(name="aggp", bufs=4, space="PSUM") as apool,
        ):
            # ---- constants ----
            ident_g = cpool.tile([P, P], f32)
            make_identity(nc, ident_g[:])
            ident = cpool.tile([P, P], f32)
            nc.vector.tensor_copy(ident[:], ident_g[:])
            iota_i = cpool.tile([P, P], mybir.dt.int32)
            nc.gpsimd.iota(iota_i[:], pattern=[[1, P]], base=0, channel_multiplier=0)
            iota_b = cpool.tile([P, P], bf16)
            nc.vector.tensor_copy(iota_b[:], iota_i[:])
            ones_g = cpool.tile([1, P], f32)
            nc.gpsimd.memset(ones_g[:], 1.0)
            ones_r = cpool.tile([1, P], bf16)
            nc.vector.tensor_copy(ones_r[:], ones_g[:])

            def load_bf16(dst, src_ap, tagname):
                t = wpool.tile(list(src_ap.shape), f32, tag="wload", name=f"wl_{tagname}")
                nc.sync.dma_start(out=t[:], in_=src_ap)
                nc.vector.tensor_copy(dst, t[:])

            wcat_g = cpool.tile([P, 2 * D], bf16)
            load_bf16(wcat_g[:, 0:D], ws[:], "ws")
            load_bf16(wcat_g[:, D:2 * D], wh[:], "wh1")
            wcat_r = cpool.tile([P, 2 * D], bf16)
            load_bf16(wcat_r[:, 0:D], wr[:], "wr")
            load_bf16(wcat_r[:, D:2 * D], wh[:], "wh2")
            wqr_b = cpool.tile([P, D], bf16)
            load_bf16(wqr_b[:], wqr[:], "wqr")
            bias_r = cpool.tile([1, D], bf16)
            load_bf16(bias_r[:], wqrb[:], "wqrb")
            wa_r = cpool.tile([1, D], bf16)
            load_bf16(wa_r[:], wa[:], "wa")

            wab_ps = ppool.tile([P, D], f32, tag="mm")
            nc.tensor.matmul(wab_ps[:], lhsT=ones_r[:], rhs=wa_r[:], start=True, stop=True)
            wab = cpool.tile([P, D], bf16)
            nc.vector.tensor_copy(wab[:], wab_ps[:])

            # ---- DRAM tables ----
            hG_slice = dpool.tile([TBL_ROWS, 2 * D], bf16)
            hG_full = dpool.tile([TBL_FULL, 2 * D], bf16, addr_space="Shared")
            hrG = dpool.tile([512, 2 * D], bf16)
            hqr_d = dpool.tile([P, D], bf16)

            # hG slice build: 49 tiles of [128, 256]
            for i in range(NWIN):
                h_t = wpool.tile([P, D], f32, tag="h_in")
                nc.sync.dma_start(out=h_t[:], in_=hid_s[i * P:(i + 1) * P, :])
                h_d = wpool.tile([P, D], f32, tag="h_dve")
                nc.vector.tensor_copy(h_d[:], h_t[:])
                tr_ps = ppool.tile([P, P], f32, tag="tr")
                nc.tensor.transpose(tr_ps[:], h_d[:], ident[:])
                hT = wpool.tile([P, P], bf16, tag="hT")
                nc.vector.tensor_copy(hT[:], tr_ps[:])
                g_ps = ppool.tile([P, 2 * D], f32, tag="mm")
                nc.tensor.matmul(g_ps[:], lhsT=hT[:], rhs=wcat_g[:], start=True, stop=True)
                g_b = wpool.tile([P, 2 * D], bf16, tag="g_out")
                nc.vector.tensor_copy(g_b[:], g_ps[:])
                nc.sync.dma_start(out=hG_slice[i * P:(i + 1) * P, :], in_=g_b[:])

            nc.gpsimd.collective_compute(
                "AllGather",
                mybir.AluOpType.bypass,
                replica_groups=[list(range(NCORES))],
                ins=[hG_slice[:]],
                outs=[hG_full[:]],
            )

            # hrG build: 4 tiles (401 rows padded to 512)
            for i in range(4):
                r_t = wpool.tile([P, D], f32, tag="h_in")
                lo = i * P
                hi = min(401, lo + P)
                if hi - lo < P:
                    nc.gpsimd.memset(r_t[:], 0.0)
                nc.sync.dma_start(out=r_t[0:hi - lo, :], in_=rela[lo:hi, :])
                r_d = wpool.tile([P, D], f32, tag="h_dve")
                nc.vector.tensor_copy(r_d[:], r_t[:])
                tr_ps = ppool.tile([P, P], f32, tag="tr")
                nc.tensor.transpose(tr_ps[:], r_d[:], ident[:])
                rT = wpool.tile([P, P], bf16, tag="hT")
                nc.vector.tensor_copy(rT[:], tr_ps[:])
                g_ps = ppool.tile([P, 2 * D], f32, tag="mm")
                nc.tensor.matmul(g_ps[:], lhsT=rT[:], rhs=wcat_r[:], start=True, stop=True)
                g_b = wpool.tile([P, 2 * D], bf16, tag="g_out")
                nc.vector.tensor_copy(g_b[:], g_ps[:])
                nc.sync.dma_start(out=hrG[i * P:(i + 1) * P, :], in_=g_b[:])

            # hqr build
            q_t = wpool.tile([P, D], f32, tag="h_in")
            nc.gpsimd.memset(q_t[:], 0.0)
            nc.sync.dma_start(out=q_t[0:64, :], in_=qrel[:])
            q_d = wpool.tile([P, D], f32, tag="h_dve")
            nc.vector.tensor_copy(q_d[:], q_t[:])
            tr_ps = ppool.tile([P, P], f32, tag="tr")
            nc.tensor.transpose(tr_ps[:], q_d[:], ident[:])
            qT = wpool.tile([P, P], bf16, tag="hT")
            nc.vector.tensor_copy(qT[:], tr_ps[:])
            q_ps = ppool.tile([P, D], f32, tag="mm")
            nc.tensor.matmul(q_ps[:], lhsT=qT[:], rhs=wqr_b[:], start=True, stop=False)
            nc.tensor.matmul(q_ps[:], lhsT=ones_r[:], rhs=bias_r[:], start=False, stop=True)
            q_b = wpool.tile([P, D], bf16, tag="g_out")
            nc.vector.tensor_copy(q_b[:], q_ps[:])
            nc.sync.dma_start(out=hqr_d[:], in_=q_b[:])

            # ---- edge index arrays resident in SBUF ----
            sub_s = cpool.tile([P, S], i16)
            nc.sync.dma_start(out=sub_s[:], in_=sub_i[:])
            rel_s = cpool.tile([P, S], i16)
            nc.sync.dma_start(out=rel_s[:], in_=rel_i[:])
            rid_s = cpool.tile([P, S], i16)
            nc.sync.dma_start(out=rid_s[:], in_=rid_i[:])
            obj_s = cpool.tile([P, ctot], f32)
            nc.sync.dma_start(out=obj_s[:], in_=obj_f[:])

            # ---- edge processing ----
            agg = {}
            for c_start, tA, tB in groups:
                T = tA + tB
                n_all = T * P
                nA = tA * P
                nB = tB * P
                s0 = c_start * P // 16

                MAXI = 1024   # dma_gather ucode limit on num_idxs per call

                def chunked_gather(dst_tile, src_ap, idxs_tile, idx_col0, t_off,
                                   n, elem):
                    done = 0
                    while done < n:
                        cn = min(MAXI, n - done)
                        ct0 = t_off + done // P
                        nc.gpsimd.dma_gather(
                            out_ap=dst_tile[:, ct0:ct0 + cn // P, :],
                            in_ap=src_ap,
                            idxs_ap=idxs_tile[:, idx_col0 + done // 16:
                                              idx_col0 + (done + cn) // 16],
                            num_idxs=cn, num_idxs_reg=cn, elem_size=elem)
                        done += cn

                g_t = wpool.tile([P, T, 2 * D], bf16, tag="g_g")
                if tA:
                    chunked_gather(g_t, hG_full[0:HALF, :], sub_s, s0, 0, nA, 2 * D)
                if tB:
                    chunked_gather(g_t, hG_full[HALF:TBL_FULL, :], sub_s,
                                   s0 + nA // 16, tA, nB, 2 * D)
                r_t = wpool.tile([P, T, 2 * D], bf16, tag="g_r")
                chunked_gather(r_t, hrG[:], rel_s, s0, 0, n_all, 2 * D)
                q_g = wpool.tile([P, T, D], bf16, tag="g_q")
                chunked_gather(q_g, hqr_d[:], rid_s, s0, 0, n_all, D)

                x1 = wpool.tile([P, T, D], bf16, tag="x1")
                nc.vector.tensor_tensor(
                    out=x1[:], in0=g_t[:, :, 0:D], in1=r_t[:, :, 0:D], op=Alu.add)
                x2 = wpool.tile([P, T, D], bf16, tag="x2")
                nc.vector.tensor_tensor(out=x2[:], in0=x1[:], in1=q_g[:], op=Alu.add)

                logit = wpool.tile([P, T], f32, tag="logit")
                dump = wpool.tile([P, D], bf16, tag="dump")
                for c in range(T):
                    nc.vector.scalar_tensor_tensor(
                        out=dump[:], in0=x2[:, c, :], scalar=0.0, in1=wab[:],
                        op0=Alu.max, op1=Alu.mult,
                        accum_out=logit[:, c:c + 1])
                alpha = wpool.tile([P, T], bf16, tag="alpha")
                nc.scalar.activation(alpha[:], logit[:], AF.Sigmoid)

                oh = wpool.tile([P, T, P], bf16, tag="oh")
                for c in range(T):
                    nc.vector.scalar_tensor_tensor(
                        out=oh[:, c, :], in0=iota_b[:],
                        scalar=obj_s[:, c_start + c:c_start + c + 1],
                        in1=alpha[:, c:c + 1].to_broadcast([P, P]),
                        op0=Alu.is_equal, op1=Alu.mult)

                msg = wpool.tile([P, T, D], bf16, tag="msg")
                nc.vector.tensor_tensor(
                    out=msg[:], in0=g_t[:, :, D:2 * D], in1=r_t[:, :, D:2 * D], op=Alu.add)

                for c in range(T):
                    ct = c_start + c
                    w = tile_window[ct]
                    if ct == first_tile[w]:
                        agg[w] = apool.tile([P, D], f32, tag="agg", name=f"agg_{w}")
                    nc.tensor.matmul(agg[w][:], lhsT=oh[:, c, :], rhs=msg[:, c, :],
                                     start=(ct == first_tile[w]),
                                     stop=(ct == last_tile[w]))
                    if ct == last_tile[w]:
                        o_t = wpool.tile([P, D], f32, tag="o_t")
                        nc.vector.tensor_scalar(
                            out=o_t[:], in0=agg[w][:], scalar1=0.0, scalar2=None,
                            op0=Alu.max)
                        nc.sync.dma_start(out=out_ext[w * P:(w + 1) * P, :], in_=o_t[:])
                        del agg[w]

    nc.compile()
    return nc


def kernel(q_rel, hidden, edges, rela_embed, Ws, Wr, Wqr_w, Wqr_b, Wa, Wh, n_node):
    from concourse.bass_utils import run_bass_kernel_spmd

    q_rel = np.asarray(q_rel)
    hidden = np.asarray(hidden, dtype=np.float32)
    edges = np.asarray(edges)
    rela_embed = np.asarray(rela_embed, dtype=np.float32)

    subs16, rels16, rids16, objs, tile_window, groups, ctot = _host_shard(edges)
    nc = _build_graph(ctot, tile_window, groups)

    hid_pad = np.zeros((TBL_FULL, D), dtype=np.float32)
    hid_pad[:N] = hidden
    qrel_sel = np.ascontiguousarray(rela_embed[np.asarray(q_rel, dtype=np.int64)])

    in_maps = []
    for k in range(NCORES):
        in_maps.append({
            "hid_s": np.ascontiguousarray(hid_pad[k * TBL_ROWS:(k + 1) * TBL_ROWS]),
            "rela": rela_embed,
            "qrel": qrel_sel,
            "ws": np.asarray(Ws, dtype=np.float32),
            "wr": np.asarray(Wr, dtype=np.float32),
            "wh": np.asarray(Wh, dtype=np.float32),
            "wqr": np.asarray(Wqr_w, dtype=np.float32),
            "wqrb": np.asarray(Wqr_b, dtype=np.float32).reshape(1, D),
            "wa": np.asarray(Wa, dtype=np.float32).reshape(1, D),
            "sub_i": subs16[k],
            "rel_i": rels16[k],
            "rid_i": rids16[k],
            "obj_f": objs[k],
        })

    res = run_bass_kernel_spmd(nc, in_maps, list(range(NCORES)))
    out = np.concatenate([res.results[k]["out"][:NPC] for k in range(NCORES)], axis=0)
    return out.astype(np.float32)


if __name__ == "__main__":
    import reference

    inputs = reference.setup_inputs()
    inputs = {k: np.asarray(v) for k, v in inputs.items()}
    got = kernel(**inputs)
    exp = np.asarray(reference.reference(**reference.setup_inputs()))
    err = np.abs(got - exp).max() / (np.abs(exp).max() + 1e-9)
    print("rel err:", err)
